# revision 1
# baseline (speedup 1.0000x reference)
"""Trainium2 Bass kernel for NnqlmCnnBasedLstm.

Math (per batch item, per input sequence q/a):
  xe = embed[idx]                      (L, D)       D = 128
  dens_t = outer(xe_t, xe_t)/(|xe_t|^2 + 1e-4)     (D, D), symmetric
  2-layer ConvLSTM over L=40 steps; each gate g:
    pre_g = conv2d([xt; h], W_g, stride=(2,1), pad=(1,1)) + b_g  on (2D, D) -> (D, D)
  c = sig(f)*c + sig(i)*tanh(cc); h = sig(o)*tanh(c)
  out = max_t h2_t  -> flatten -> concat(q,a) -> linear(2) -> log_softmax

Device strategy (8 cores, data parallel over B=32 -> 4 items/core, each with a
q-chain and an a-chain = 8 chains/core):
  * State kept TRANSPOSED: tiles are (w partitions, j free).  The density is
    symmetric so layer-1 inputs need no transpose.
  * conv: out_T[w, j] = sum_{dh,dw} W[dh,dw] * inp_T[w-1+dw, 2j-1+dh].
    For each dh this is a 3-diagonal Toeplitz band matrix (over w) applied via
    the TensorEngine, with the (2j-1+dh) selection expressed as a stride-2
    free-axis access pattern on the moving operand.  4 gates x 4 dh matmuls
    accumulate in PSUM; all 8 chains batched in the moving free dim.
  * sigmoid/tanh (+conv bias) on ScalarE reading PSUM; cell updates on VectorE;
    densities as rank-1 (K=1) outer-product matmuls on the TensorEngine.
  * Embedding gather, final linear + log_softmax on host (tiny).
"""

import os
import sys

import numpy as np

for _p in ("/opt/trn_rl_repo", "/root/.axon_site/_ro/trn_rl_repo"):
    if os.path.isdir(_p) and _p not in sys.path:
        sys.path.insert(0, _p)

B, L, D, V, NL = 32, 40, 128, 32000, 2
NCORES = 8
CH = 8            # chains per core: 4 batch items x {q, a}
SEG = 2 * D + 2   # per-chain column span in the input tile: [0]=0, [1..128]=x, [129..256]=h, [257]=0
NF = CH * SEG
NV = L * CH       # 320 embedding vectors per core
NVP = 384         # padded to a multiple of 128

_CACHE = {}


def _build_nc(L=L):
    import concourse.bass as bass
    import concourse.bacc as bacc
    import concourse.mybir as mybir
    from concourse import tile

    f32 = mybir.dt.float32
    AF = mybir.ActivationFunctionType
    ALU = mybir.AluOpType

    nc = bacc.Bacc(None, target_bir_lowering=False)

    xey_d = nc.dram_tensor("xey", (L, 1, CH * D), f32, kind="ExternalInput")
    st_d = nc.dram_tensor("st", (NL * 4 * 4, D, D), f32, kind="ExternalInput")
    bias_d = nc.dram_tensor("bias", (D, NL * 4), f32, kind="ExternalInput")
    out_d = nc.dram_tensor("mp_out", (D, CH * D), f32, kind="ExternalOutput")

    with tile.TileContext(nc) as tc:
        with (
            tc.tile_pool(name="const", bufs=1) as constp,
            tc.tile_pool(name="state", bufs=1) as statep,
            tc.tile_pool(name="inp", bufs=2) as inpp,
            tc.tile_pool(name="gate", bufs=2) as gatep,
            tc.tile_pool(name="psum", bufs=1, space="PSUM") as psump,
        ):
            # ---- constants ----
            stT = constp.tile([D, NL * 4 * 4 * D], f32, tag="stT")
            for i in range(NL * 4 * 4):
                nc.sync.dma_start(stT[:, i * D:(i + 1) * D], st_d[i])

            bias = constp.tile([D, NL * 4], f32, tag="bias")
            nc.sync.dma_start(bias[:], bias_d[:])

            # ---- persistent state ----
            c_l = [statep.tile([D, CH * D], f32, tag=f"c{l}", name=f"c{l}") for l in range(NL)]
            mp = statep.tile([D, CH * D], f32, tag="mp")
            for l in range(NL):
                nc.vector.memset(c_l[l][:], 0.0)
            nc.vector.memset(mp[:], -1e30)

            def seg3(t):  # (p, s, c) view of an input tile
                return t[:].rearrange("p (s c) -> p s c", s=CH)

            def seg4(t):  # (p, s, c2, two) parity view for stride-2 j access
                return t[:].rearrange("p (s c two) -> p s c two", s=CH, two=2)

            def new_inp(tag):
                t = inpp.tile([D, NF], f32, tag=tag, name=tag)
                # zero the pad columns (0 and 257 of each chain segment)
                v = t[:].rearrange("p (s c) -> p s c", s=CH)
                nc.gpsimd.memset(v[:, :, 0:1], 0.0)
                nc.gpsimd.memset(v[:, :, SEG - 1:SEG], 0.0)
                return t

            def outers(t_next, dst_tile):
                """Rank-1 matmuls: densities for step t_next -> x-part of dst_tile."""
                stage = gatep.tile([1, CH * D], f32, tag="xstage", name="xstage")
                nc.sync.dma_start(stage[:], xey_d[t_next])
                po = psump.tile([D, CH * D], f32, tag="pf", name="po")
                for s in range(CH):
                    vec = stage[0:1, s * D:(s + 1) * D]
                    nc.tensor.matmul(
                        po[:, s * D:(s + 1) * D],
                        vec, vec,
                        start=True, stop=True,
                    )
                v3 = seg3(dst_tile)
                for hf in range(2):
                    nc.scalar.activation(v3[:, hf * 4:(hf + 1) * 4, 1:1 + D],
                                         po[:, hf * 512:(hf + 1) * 512], AF.Copy)

            cur = [None, None]
            cur[0] = new_inp("inp0")
            cur[1] = new_inp("inp1")
            nc.gpsimd.memset(seg3(cur[0])[:, :, 129:129 + D], 0.0)   # h1_{-1} = 0
            nc.gpsimd.memset(seg3(cur[1])[:, :, 129:129 + D], 0.0)   # h2_{-1} = 0
            outers(0, cur[0])

            GTAG = ["pf", "pi", "po", "pc"]
            for t in range(L):
                nxt = [None, None]
                nxt[0] = new_inp("inp0") if t + 1 < L else None
                nxt[1] = new_inp("inp1") if t + 1 < L else None

                for l in range(NL):
                    inp = cur[l]
                    i4 = seg4(inp)
                    # --- gate pre-activations: 4 gates x 4 dh band matmuls ---
                    ps = [psump.tile([D, CH * D], f32, tag=GTAG[g], name=GTAG[g]) for g in range(4)]
                    for g in range(4):
                        for half in range(2):
                            for dh in range(4):
                                idx = (l * 4 + g) * 4 + dh
                                rhs = i4[:, half * 4:(half + 1) * 4,
                                         dh // 2: dh // 2 + D, dh % 2]
                                nc.tensor.matmul(
                                    ps[g][:, half * 512:(half + 1) * 512],
                                    stT[:, idx * D:(idx + 1) * D],
                                    rhs,
                                    start=(dh == 0), stop=(dh == 3),
                                )
                    # --- activations (bias folded in) ---
                    fg = gatep.tile([D, CH * D], f32, tag="fg")
                    ig = gatep.tile([D, CH * D], f32, tag="ig")
                    og = gatep.tile([D, CH * D], f32, tag="og")
                    cs = gatep.tile([D, CH * D], f32, tag="cs")
                    for g, dst in enumerate((fg, ig, og)):
                        nc.scalar.activation(dst[:], ps[g][:], AF.Sigmoid,
                                             bias=bias[:, l * 4 + g: l * 4 + g + 1])
                    nc.scalar.activation(cs[:], ps[3][:], AF.Tanh,
                                         bias=bias[:, l * 4 + 3: l * 4 + 4])
                    # --- cell update ---
                    t1 = gatep.tile([D, CH * D], f32, tag="t1")
                    t2 = gatep.tile([D, CH * D], f32, tag="t2")
                    nc.vector.tensor_mul(t1[:], fg[:], c_l[l][:])
                    nc.vector.tensor_mul(t2[:], ig[:], cs[:])
                    nc.vector.tensor_add(c_l[l][:], t1[:], t2[:])
                    th = gatep.tile([D, CH * D], f32, tag="th")
                    nc.scalar.activation(th[:], c_l[l][:], AF.Tanh)
                    # --- h = og * tanh(c): route to consumers ---
                    if l == 0:
                        # h1_t -> x-part of layer-2 input (this step)
                        nc.vector.tensor_mul(seg3(cur[1])[:, :, 1:1 + D], og[:], th[:])
                        if nxt[0] is not None:
                            # copy h1_t -> h-part of next layer-1 input
                            nc.gpsimd.tensor_copy(
                                seg3(nxt[0])[:, :, 129:129 + D],
                                seg3(cur[1])[:, :, 1:1 + D],
                            )
                    else:
                        if nxt[1] is not None:
                            h2dst = seg3(nxt[1])[:, :, 129:129 + D]
                            nc.vector.tensor_mul(h2dst, og[:], th[:])
                            nc.vector.tensor_tensor(mp[:], mp[:], h2dst, op=ALU.max)
                        else:
                            h2 = gatep.tile([D, CH * D], f32, tag="h2last")
                            nc.vector.tensor_mul(h2[:], og[:], th[:])
                            nc.vector.tensor_tensor(mp[:], mp[:], h2[:], op=ALU.max)

                if nxt[0] is not None:
                    outers(t + 1, nxt[0])
                cur = nxt

            nc.sync.dma_start(out_d[:], mp[:])

    nc.compile()
    return nc


def _prep_core_inputs(xe_y, st, bias_arr, core):
    """xe_y: (B, 2, L, D) sqrt-normalized embeddings (axis1: 0=q, 1=a)."""
    sl = slice(4 * core, 4 * core + 4)
    # chains: s=0..3 -> q items, s=4..7 -> a items
    ch = np.concatenate([xe_y[sl, 0], xe_y[sl, 1]], axis=0)    # (8, L, D)
    xey = np.ascontiguousarray(ch.transpose(1, 0, 2)).reshape(L, 1, CH * D)
    return {"xey": xey, "st": st, "bias": bias_arr}


def kernel(q, a, embed, conv_w, conv_b, lin_w, lin_b):
    from concourse import bass_utils

    q = np.asarray(q); a = np.asarray(a)
    embed = np.asarray(embed, np.float32)
    conv_w = np.asarray(conv_w, np.float32)
    conv_b = np.asarray(conv_b, np.float32)
    lin_w = np.asarray(lin_w, np.float32)
    lin_b = np.asarray(lin_b, np.float32)

    # host: embedding gather + density normalization factors
    idx = np.stack([q, a], axis=1).astype(np.int64)            # (B, 2, L)
    xe = embed[idx].astype(np.float64)                         # (B, 2, L, D)
    dot = np.sum(xe * xe, axis=-1, keepdims=True) + 1e-4
    xe_y = (xe / np.sqrt(dot)).astype(np.float32)

    # host: Toeplitz band stationaries  lhsT[(l,g,dh)] = B^T,
    # B[w, w'] = W[dh, w'-w+1]  (3 diagonals)
    st = np.zeros((NL * 4 * 4, D, D), np.float32)
    for l in range(NL):
        for g in range(4):
            W = conv_w[l, g, 0, 0]                             # (4, 3)
            for dh in range(4):
                Bm = sum(W[dh, dw] * np.eye(D, k=dw - 1) for dw in range(3))
                st[(l * 4 + g) * 4 + dh] = Bm.T.astype(np.float32)
    bias_arr = np.tile(conv_b.reshape(1, -1), (D, 1)).astype(np.float32)

    if "nc" not in _CACHE:
        _CACHE["nc"] = _build_nc()
    nc = _CACHE["nc"]

    in_maps = [_prep_core_inputs(xe_y, st, bias_arr, i) for i in range(NCORES)]
    _CACHE["in_maps"] = in_maps
    res = bass_utils.run_bass_kernel_spmd(nc, in_maps, core_ids=list(range(NCORES)))

    # host: unshard + final linear + log_softmax
    q_p = np.zeros((B, D * D), np.float32)
    a_p = np.zeros((B, D * D), np.float32)
    for i in range(NCORES):
        out = res.results[i]["mp_out"]                         # (D w, CH*D)
        for s in range(CH):
            mp_T = out[:, s * D:(s + 1) * D]                   # (w, j)
            flat = np.ascontiguousarray(mp_T.T).reshape(-1)    # j-major
            if s < 4:
                q_p[4 * i + s] = flat
            else:
                a_p[4 * i + s - 4] = flat
    qa = np.concatenate([q_p, a_p], axis=1)
    score = qa @ lin_w.T + lin_b
    m = score.max(axis=1, keepdims=True)
    ls = score - m
    lse = np.log(np.exp(ls).sum(axis=1, keepdims=True))
    return (ls - lse).astype(np.float32)



# revision 10
# speedup vs baseline: 1.8113x; 1.8113x over previous
"""Trainium2 Bass kernel for NnqlmCnnBasedLstm.

Math (per batch item, per input sequence q/a):
  xe = embed[idx]                      (L, D)       D = 128
  dens_t = outer(xe_t, xe_t)/(|xe_t|^2 + 1e-4)     (D, D), symmetric
  2-layer ConvLSTM over L=40 steps; each gate g:
    pre_g = conv2d([xt; h], W_g, stride=(2,1), pad=(1,1)) + b_g  on (2D, D) -> (D, D)
  c = sig(f)*c + sig(i)*tanh(cc); h = sig(o)*tanh(c)
  out = max_t h2_t  -> flatten -> concat(q,a) -> linear(2) -> log_softmax

Device strategy (8 cores, data parallel over B=32 -> 4 items/core, each with a
q-chain and an a-chain = 8 chains/core):
  * State kept TRANSPOSED: tiles are (w partitions, j free).  The density is
    symmetric so layer-1 inputs need no transpose.
  * conv: out_T[w, j] = sum_{dh,dw} W[dh,dw] * inp_T[w-1+dw, 2j-1+dh].
    For each dh this is a 3-diagonal Toeplitz band matrix (over w) applied via
    the TensorEngine, with the (2j-1+dh) selection expressed as a stride-2
    free-axis access pattern on the moving operand.  4 gates x 4 dh matmuls
    accumulate in PSUM; all 8 chains batched in the moving free dim.
  * sigmoid/tanh (+conv bias) on ScalarE reading PSUM; cell updates on VectorE;
    densities as rank-1 (K=1) outer-product matmuls on the TensorEngine.
  * Embedding gather, final linear + log_softmax on host (tiny).
"""

import os
import sys

import numpy as np

for _p in ("/opt/trn_rl_repo", "/root/.axon_site/_ro/trn_rl_repo"):
    if os.path.isdir(_p) and _p not in sys.path:
        sys.path.insert(0, _p)

B, L, D, V, NL = 32, 40, 128, 32000, 2
NCORES = 8
CH = 8            # chains per core: 4 batch items x {q, a}
SEG = 2 * D + 2   # per-chain column span in the input tile: [0]=0, [1..128]=x, [129..256]=h, [257]=0
NF = CH * SEG
NV = L * CH       # 320 embedding vectors per core
NVP = 384         # padded to a multiple of 128

_CACHE = {}


def _build_nc(L=L):
    import concourse.bass as bass
    import concourse.bacc as bacc
    import concourse.mybir as mybir
    from concourse import tile

    f32 = mybir.dt.float32
    f32r = mybir.dt.float32r
    AF = mybir.ActivationFunctionType
    ALU = mybir.AluOpType

    nc = bacc.Bacc(None, target_bir_lowering=False)

    xey_d = nc.dram_tensor("xey", (L, 1, CH * D), f32, kind="ExternalInput")
    st_d = nc.dram_tensor("st", (NL * 4 * 4, D, D), f32r, kind="ExternalInput")
    bias_d = nc.dram_tensor("bias", (D, NL * 4), f32, kind="ExternalInput")
    out_d = nc.dram_tensor("mp_out", (D, CH * D), f32, kind="ExternalOutput")

    with tile.TileContext(nc) as tc:
        with (
            tc.tile_pool(name="const", bufs=1) as constp,
            tc.tile_pool(name="state", bufs=1) as statep,
            tc.tile_pool(name="inp", bufs=2) as inpp,
            tc.tile_pool(name="gate", bufs=2) as gatep,
            tc.tile_pool(name="psum", bufs=1, space="PSUM") as psump,
        ):
            # ---- constants ----
            stT = constp.tile([D, NL * 4 * 4 * D], f32r, tag="stT")
            for i in range(NL * 4 * 4):
                nc.sync.dma_start(stT[:, i * D:(i + 1) * D], st_d[i])

            bias = constp.tile([D, NL * 4], f32, tag="bias")
            nc.sync.dma_start(bias[:], bias_d[:])

            # ---- persistent state ----
            c_l = [statep.tile([D, CH * D], f32, tag=f"c{l}", name=f"c{l}") for l in range(NL)]
            mp = statep.tile([D, CH * D], f32, tag="mp")
            for l in range(NL):
                nc.vector.memset(c_l[l][:], 0.0)
            nc.vector.memset(mp[:], -1e30)

            def seg3(t):  # (p, s, c) view of an input tile
                return t[:].rearrange("p (s c) -> p s c", s=CH)

            def seg4(t):  # (p, s, c2, two) parity view for stride-2 j access
                return t[:].rearrange("p (s c two) -> p s c two", s=CH, two=2)

            def new_inp(tag):
                t = inpp.tile([D, NF], f32r, tag=tag, name=tag)
                # zero the pad columns (0 and 257 of each chain segment)
                v = t[:].rearrange("p (s c) -> p s c", s=CH)
                nc.gpsimd.memset(v[:, :, 0:1].bitcast(f32), 0.0)
                nc.gpsimd.memset(v[:, :, SEG - 1:SEG].bitcast(f32), 0.0)
                return t

            def outers(t_next, dst_tile):
                """Rank-1 matmuls: densities for step t_next -> x-part of dst_tile."""
                stage = gatep.tile([1, CH * D], f32, tag="xstage", name="xstage")
                nc.sync.dma_start(stage[:], xey_d[t_next])
                po = psump.tile([D, CH * D], f32, tag="pf", name="po")
                for s in range(CH):
                    vec = stage[0:1, s * D:(s + 1) * D]
                    nc.tensor.matmul(
                        po[:, s * D:(s + 1) * D],
                        vec, vec,
                        start=True, stop=True,
                    )
                v3 = seg3(dst_tile)
                for hf in range(2):
                    nc.scalar.activation(v3[:, hf * 4:(hf + 1) * 4, 1:1 + D],
                                         po[:, hf * 512:(hf + 1) * 512], AF.Copy)

            cur = [None, None]
            cur[0] = new_inp("inp0")
            cur[1] = new_inp("inp1")
            nc.gpsimd.memset(seg3(cur[0])[:, :, 129:129 + D].bitcast(f32), 0.0)
            nc.gpsimd.memset(seg3(cur[1])[:, :, 129:129 + D].bitcast(f32), 0.0)
            outers(0, cur[0])

            GTAG = ["pf", "pi", "po", "pc"]
            for t in range(L):
                nxt = [None, None]
                nxt[0] = new_inp("inp0") if t + 1 < L else None
                nxt[1] = new_inp("inp1") if t + 1 < L else None

                for l in range(NL):
                    inp = cur[l]
                    i4 = seg4(inp)
                    # --- gate pre-activations: 4 gates x 4 dh band matmuls ---
                    ps = [psump.tile([D, CH * D], f32, tag=GTAG[g], name=GTAG[g]) for g in range(4)]
                    for g in range(4):
                        for half in range(2):
                            for dh in range(4):
                                idx = (l * 4 + g) * 4 + dh
                                rhs = i4[:, half * 4:(half + 1) * 4,
                                         dh // 2: dh // 2 + D, dh % 2]
                                nc.tensor.matmul(
                                    ps[g][:, half * 512:(half + 1) * 512],
                                    stT[:, idx * D:(idx + 1) * D],
                                    rhs,
                                    start=(dh == 0), stop=(dh == 3),
                                )
                    # --- activations (bias folded in) ---
                    fg = gatep.tile([D, CH * D], f32, tag="fg")
                    ig = gatep.tile([D, CH * D], f32, tag="ig")
                    og = gatep.tile([D, CH * D], f32, tag="og")
                    cs = gatep.tile([D, CH * D], f32, tag="cs")
                    for g, dst in enumerate((fg, ig, og)):
                        nc.scalar.activation(dst[:], ps[g][:], AF.Sigmoid,
                                             bias=bias[:, l * 4 + g: l * 4 + g + 1])
                    nc.scalar.activation(cs[:], ps[3][:], AF.Tanh,
                                         bias=bias[:, l * 4 + 3: l * 4 + 4])
                    # --- cell update ---
                    t1 = gatep.tile([D, CH * D], f32, tag="t1")
                    t2 = gatep.tile([D, CH * D], f32, tag="t2")
                    nc.vector.tensor_mul(t1[:], fg[:], c_l[l][:])
                    nc.vector.tensor_mul(t2[:], ig[:], cs[:])
                    nc.vector.tensor_add(c_l[l][:], t1[:], t2[:])
                    th = gatep.tile([D, CH * D], f32, tag="th")
                    nc.scalar.activation(th[:], c_l[l][:], AF.Tanh)
                    # --- h = og * tanh(c): route to consumers ---
                    if l == 0:
                        # h1_t -> x-part of layer-2 input (this step)
                        nc.vector.tensor_mul(seg3(cur[1])[:, :, 1:1 + D], og[:], th[:])
                        if nxt[0] is not None:
                            # copy h1_t -> h-part of next layer-1 input
                            nc.gpsimd.tensor_copy(
                                seg3(nxt[0])[:, :, 129:129 + D],
                                seg3(cur[1])[:, :, 1:1 + D],
                            )
                    else:
                        if nxt[1] is not None:
                            h2dst = seg3(nxt[1])[:, :, 129:129 + D]
                            nc.vector.tensor_mul(h2dst, og[:], th[:])
                            nc.vector.tensor_tensor(mp[:], mp[:], h2dst.bitcast(f32),
                                                    op=ALU.max)
                        else:
                            h2 = gatep.tile([D, CH * D], f32, tag="h2last")
                            nc.vector.tensor_mul(h2[:], og[:], th[:])
                            nc.vector.tensor_tensor(mp[:], mp[:], h2[:], op=ALU.max)

                if nxt[0] is not None:
                    outers(t + 1, nxt[0])
                cur = nxt

            nc.sync.dma_start(out_d[:], mp[:])

    nc.compile()
    return nc


def _prep_core_inputs(xe_y, st, bias_arr, core):
    """xe_y: (B, 2, L, D) sqrt-normalized embeddings (axis1: 0=q, 1=a)."""
    sl = slice(4 * core, 4 * core + 4)
    # chains: s=0..3 -> q items, s=4..7 -> a items
    ch = np.concatenate([xe_y[sl, 0], xe_y[sl, 1]], axis=0)    # (8, L, D)
    xey = np.ascontiguousarray(ch.transpose(1, 0, 2)).reshape(L, 1, CH * D)
    return {"xey": xey, "st": st, "bias": bias_arr}


def kernel(q, a, embed, conv_w, conv_b, lin_w, lin_b):
    from concourse import bass_utils

    q = np.asarray(q); a = np.asarray(a)
    embed = np.asarray(embed, np.float32)
    conv_w = np.asarray(conv_w, np.float32)
    conv_b = np.asarray(conv_b, np.float32)
    lin_w = np.asarray(lin_w, np.float32)
    lin_b = np.asarray(lin_b, np.float32)

    # host: embedding gather + density normalization factors
    idx = np.stack([q, a], axis=1).astype(np.int64)            # (B, 2, L)
    xe = embed[idx].astype(np.float64)                         # (B, 2, L, D)
    dot = np.sum(xe * xe, axis=-1, keepdims=True) + 1e-4
    xe_y = (xe / np.sqrt(dot)).astype(np.float32)

    # host: Toeplitz band stationaries  lhsT[(l,g,dh)] = B^T,
    # B[w, w'] = W[dh, w'-w+1]  (3 diagonals)
    st = np.zeros((NL * 4 * 4, D, D), np.float32)
    for l in range(NL):
        for g in range(4):
            W = conv_w[l, g, 0, 0]                             # (4, 3)
            for dh in range(4):
                Bm = sum(W[dh, dw] * np.eye(D, k=dw - 1) for dw in range(3))
                st[(l * 4 + g) * 4 + dh] = Bm.T.astype(np.float32)
    bias_arr = np.tile(conv_b.reshape(1, -1), (D, 1)).astype(np.float32)

    if "nc" not in _CACHE:
        _CACHE["nc"] = _build_nc()
    nc = _CACHE["nc"]

    in_maps = [_prep_core_inputs(xe_y, st, bias_arr, i) for i in range(NCORES)]
    _CACHE["in_maps"] = in_maps
    res = bass_utils.run_bass_kernel_spmd(nc, in_maps, core_ids=list(range(NCORES)))

    # host: unshard + final linear + log_softmax
    q_p = np.zeros((B, D * D), np.float32)
    a_p = np.zeros((B, D * D), np.float32)
    for i in range(NCORES):
        out = res.results[i]["mp_out"]                         # (D w, CH*D)
        for s in range(CH):
            mp_T = out[:, s * D:(s + 1) * D]                   # (w, j)
            flat = np.ascontiguousarray(mp_T.T).reshape(-1)    # j-major
            if s < 4:
                q_p[4 * i + s] = flat
            else:
                a_p[4 * i + s - 4] = flat
    qa = np.concatenate([q_p, a_p], axis=1)
    score = qa @ lin_w.T + lin_b
    m = score.max(axis=1, keepdims=True)
    ls = score - m
    lse = np.log(np.exp(ls).sum(axis=1, keepdims=True))
    return (ls - lse).astype(np.float32)



# revision 11
# speedup vs baseline: 2.7818x; 1.5359x over previous
"""Trainium2 Bass kernel for NnqlmCnnBasedLstm.

Math (per batch item, per input sequence q/a):
  xe = embed[idx]                      (L, D)       D = 128
  dens_t = outer(xe_t, xe_t)/(|xe_t|^2 + 1e-4)     (D, D), symmetric
  2-layer ConvLSTM over L=40 steps; each gate g:
    pre_g = conv2d([xt; h], W_g, stride=(2,1), pad=(1,1)) + b_g  on (2D, D) -> (D, D)
  c = sig(f)*c + ig*tanh(cc); h = og*tanh(c)
  out = max_t h2_t  -> flatten -> concat(q,a) -> linear(2) -> log_softmax

Device strategy (8 cores, data parallel over B=32 -> 4 items/core, each with a
q-chain and an a-chain = 8 chains/core):
  * State kept TRANSPOSED: tiles are (w partitions, j free).  Densities are
    symmetric, precomputed on HOST, and DMAed per step (DMA engines are idle).
  * conv: out_T[w, j] = sum_{dh,dw} W[dh,dw] * inp_T[w-1+dw, 2j-1+dh].
    For each dh this is a 3-diagonal Toeplitz band matrix (over w) applied via
    the TensorEngine (bf16 stationary+moving = 1 cycle/row, half-cost
    ldweights), with the (2j-1+dh) selection expressed as a stride-2 free-axis
    access pattern on the moving operand.  4 gates x 4 dh matmuls accumulate
    in fp32 PSUM; all 8 chains batched in the moving free dim.
  * The two ConvLSTM layers are software-pipelined with a 1-step skew
    (emit P0(t) then P1(t-1)): the PE runs one layer's matmul burst while the
    other layer's Activation/Vector chain completes, so the PE never stalls
    and stays at its top p-state.
  * sigmoid/tanh (+conv bias) on ScalarE reading PSUM; cell updates on
    VectorE in fp32; h writes round to bf16; h1 fan-out copy on Pool.
  * Embedding gather, final linear + log_softmax on host (tiny).
"""

import os
import sys

import numpy as np

for _p in ("/opt/trn_rl_repo", "/root/.axon_site/_ro/trn_rl_repo"):
    if os.path.isdir(_p) and _p not in sys.path:
        sys.path.insert(0, _p)

B, L, D, V, NL = 32, 40, 128, 32000, 2
NCORES = 8
CH = 8            # chains per core: 4 batch items x {q, a}
SEG = 2 * D + 2   # per-chain column span in the input tile: [0]=0, [1..128]=x, [129..256]=h, [257]=0
NF = CH * SEG

_CACHE = {}


def _build_nc(L=L):
    import concourse.bass as bass
    import concourse.bacc as bacc
    import concourse.mybir as mybir
    from concourse import tile

    f32 = mybir.dt.float32
    bf16 = mybir.dt.bfloat16
    AF = mybir.ActivationFunctionType
    ALU = mybir.AluOpType

    nc = bacc.Bacc(None, target_bir_lowering=False)

    dens_d = nc.dram_tensor("dens", (L, D, CH * D), bf16, kind="ExternalInput")
    st_d = nc.dram_tensor("st", (NL * 4 * 4, D, D), bf16, kind="ExternalInput")
    bias_d = nc.dram_tensor("bias", (D, NL * 4), f32, kind="ExternalInput")
    out_d = nc.dram_tensor("mp_out", (D, CH * D), bf16, kind="ExternalOutput")

    with tile.TileContext(nc) as tc:
        with (
            tc.tile_pool(name="const", bufs=1) as constp,
            tc.tile_pool(name="state", bufs=1) as statep,
            tc.tile_pool(name="gate", bufs=2) as gatep,
            tc.tile_pool(name="psum", bufs=1, space="PSUM") as psump,
        ):
            # ---- constants ----
            stT = constp.tile([D, NL * 4 * 4 * D], bf16, tag="stT")
            for i in range(NL * 4 * 4):
                nc.sync.dma_start(stT[:, i * D:(i + 1) * D], st_d[i])

            bias = constp.tile([D, NL * 4], f32, tag="bias")
            nc.sync.dma_start(bias[:], bias_d[:])

            # ---- persistent state ----
            c_l = [statep.tile([D, CH * D], f32, tag=f"c{l}", name=f"c{l}") for l in range(NL)]
            mp = statep.tile([D, CH * D], bf16, tag="mp")
            for l in range(NL):
                nc.vector.memset(c_l[l][:], 0.0)
            nc.vector.memset(mp[:], -1e30)

            def seg3(t):  # (p, s, c) view of an input tile
                return t[:].rearrange("p (s c) -> p s c", s=CH)

            def seg4(t):  # (p, s, c2, two) parity view for stride-2 j access
                return t[:].rearrange("p (s c two) -> p s c two", s=CH, two=2)

            # ping-pong input tiles per layer: I0[t%2] = layer-1 input at t,
            # I1[t%2] = layer-2 input at t.
            I0 = [statep.tile([D, NF], bf16, tag=f"I0{p}", name=f"I0{p}") for p in range(2)]
            I1 = [statep.tile([D, NF], bf16, tag=f"I1{p}", name=f"I1{p}") for p in range(2)]
            for t_ in I0 + I1:
                v = seg3(t_)
                nc.gpsimd.memset(v[:, :, 0:1], 0.0)
                nc.gpsimd.memset(v[:, :, SEG - 1:SEG], 0.0)
            nc.gpsimd.memset(seg3(I0[0])[:, :, 129:129 + D], 0.0)  # h1_{-1} = 0
            nc.gpsimd.memset(seg3(I1[0])[:, :, 129:129 + D], 0.0)  # h2_{-1} = 0

            # density prefetch for steps 0 and 1
            nc.sync.dma_start(seg3(I0[0])[:, :, 1:1 + D], dens_d[0])
            nc.sync.dma_start(seg3(I0[1])[:, :, 1:1 + D], dens_d[1])

            GTAG = ["pf", "pi", "po", "pc"]

            def emit_P(l, t):
                """One layer-step: matmul burst -> activations -> cell update
                -> route h to consumers."""
                inp = I0[t % 2] if l == 0 else I1[t % 2]
                i4 = seg4(inp)
                ps = [psump.tile([D, CH * D], f32, tag=GTAG[g], name=GTAG[g])
                      for g in range(4)]
                for g in range(4):
                    for half in range(2):
                        for dh in range(4):
                            idx = (l * 4 + g) * 4 + dh
                            rhs = i4[:, half * 4:(half + 1) * 4,
                                     dh // 2: dh // 2 + D, dh % 2]
                            nc.tensor.matmul(
                                ps[g][:, half * 512:(half + 1) * 512],
                                stT[:, idx * D:(idx + 1) * D],
                                rhs,
                                start=(dh == 0), stop=(dh == 3),
                            )
                # --- activations (bias folded in) ---
                fg = gatep.tile([D, CH * D], f32, tag="fg")
                ig = gatep.tile([D, CH * D], f32, tag="ig")
                og = gatep.tile([D, CH * D], f32, tag="og")
                cs = gatep.tile([D, CH * D], f32, tag="cs")
                for g, dst in enumerate((fg, ig, og)):
                    nc.scalar.activation(dst[:], ps[g][:], AF.Sigmoid,
                                         bias=bias[:, l * 4 + g: l * 4 + g + 1])
                nc.scalar.activation(cs[:], ps[3][:], AF.Tanh,
                                     bias=bias[:, l * 4 + 3: l * 4 + 4])
                # --- cell update ---
                t1 = gatep.tile([D, CH * D], f32, tag="t1")
                t2 = gatep.tile([D, CH * D], f32, tag="t2")
                nc.vector.tensor_mul(t1[:], fg[:], c_l[l][:])
                nc.vector.tensor_mul(t2[:], ig[:], cs[:])
                nc.vector.tensor_add(c_l[l][:], t1[:], t2[:])
                th = gatep.tile([D, CH * D], f32, tag="th")
                nc.scalar.activation(th[:], c_l[l][:], AF.Tanh)
                # --- h = og * tanh(c): route to consumers (rounds to bf16) ---
                if l == 0:
                    # h1_t -> x-part of layer-2 input (this step)
                    h1dst = seg3(I1[t % 2])[:, :, 1:1 + D]
                    nc.vector.tensor_mul(h1dst, og[:], th[:])
                    if t + 1 < L:
                        # copy h1_t -> h-part of next layer-1 input
                        nc.gpsimd.tensor_copy(
                            seg3(I0[(t + 1) % 2])[:, :, 129:129 + D], h1dst)
                else:
                    if t + 1 < L:
                        h2dst = seg3(I1[(t + 1) % 2])[:, :, 129:129 + D]
                        nc.vector.tensor_mul(h2dst, og[:], th[:])
                        nc.vector.tensor_tensor(mp[:], mp[:], h2dst, op=ALU.max)
                    else:
                        h2 = gatep.tile([D, CH * D], bf16, tag="h2last")
                        nc.vector.tensor_mul(h2[:], og[:], th[:])
                        nc.vector.tensor_tensor(mp[:], mp[:], h2[:], op=ALU.max)

            # skewed schedule: PE alternates between the two layers' bursts
            emit_P(0, 0)
            for t in range(1, L):
                if t + 1 < L:
                    nc.sync.dma_start(seg3(I0[(t + 1) % 2])[:, :, 1:1 + D],
                                      dens_d[t + 1])
                emit_P(0, t)
                emit_P(1, t - 1)
            emit_P(1, L - 1)

            nc.sync.dma_start(out_d[:], mp[:])

    nc.compile()
    return nc


def _prep_core_inputs(dens_all, st, bias_arr, core):
    """dens_all: (B, 2, L, D, D) bf16 densities (axis1: 0=q, 1=a)."""
    import ml_dtypes
    sl = slice(4 * core, 4 * core + 4)
    ch = np.concatenate([dens_all[sl, 0], dens_all[sl, 1]], axis=0)  # (8, L, D, D)
    # dens[t, w, s*128+j] = ch[s, t, w, j]
    dens = np.ascontiguousarray(ch.transpose(1, 2, 0, 3)).reshape(L, D, CH * D)
    return {"dens": dens, "st": st, "bias": bias_arr}


def kernel(q, a, embed, conv_w, conv_b, lin_w, lin_b):
    import ml_dtypes
    from concourse import bass_utils

    bf16 = ml_dtypes.bfloat16
    q = np.asarray(q); a = np.asarray(a)
    embed = np.asarray(embed, np.float32)
    conv_w = np.asarray(conv_w, np.float32)
    conv_b = np.asarray(conv_b, np.float32)
    lin_w = np.asarray(lin_w, np.float32)
    lin_b = np.asarray(lin_b, np.float32)

    # host: embedding gather + density (normalized outer products)
    idx = np.stack([q, a], axis=1).astype(np.int64)            # (B, 2, L)
    xe = embed[idx].astype(np.float64)                         # (B, 2, L, D)
    dot = np.sum(xe * xe, axis=-1, keepdims=True) + 1e-4
    xe_y = (xe / np.sqrt(dot)).astype(np.float32)
    dens_all = np.einsum('bslw,bslj->bslwj', xe_y, xe_y).astype(bf16)

    # host: Toeplitz band stationaries  lhsT[(l,g,dh)] = B^T,
    # B[w, w'] = W[dh, w'-w+1]  (3 diagonals)
    st = np.zeros((NL * 4 * 4, D, D), np.float32)
    for l in range(NL):
        for g in range(4):
            W = conv_w[l, g, 0, 0]                             # (4, 3)
            for dh in range(4):
                Bm = sum(W[dh, dw] * np.eye(D, k=dw - 1) for dw in range(3))
                st[(l * 4 + g) * 4 + dh] = Bm.T.astype(np.float32)
    st = st.astype(bf16)
    bias_arr = np.tile(conv_b.reshape(1, -1), (D, 1)).astype(np.float32)

    if "nc" not in _CACHE:
        _CACHE["nc"] = _build_nc()
    nc = _CACHE["nc"]

    in_maps = [_prep_core_inputs(dens_all, st, bias_arr, i) for i in range(NCORES)]
    _CACHE["in_maps"] = in_maps
    res = bass_utils.run_bass_kernel_spmd(nc, in_maps, core_ids=list(range(NCORES)))

    # host: unshard + final linear + log_softmax
    q_p = np.zeros((B, D * D), np.float32)
    a_p = np.zeros((B, D * D), np.float32)
    for i in range(NCORES):
        out = np.asarray(res.results[i]["mp_out"]).astype(np.float32)  # (D w, CH*D)
        for s in range(CH):
            mp_T = out[:, s * D:(s + 1) * D]                   # (w, j)
            flat = np.ascontiguousarray(mp_T.T).reshape(-1)    # j-major
            if s < 4:
                q_p[4 * i + s] = flat
            else:
                a_p[4 * i + s - 4] = flat
    qa = np.concatenate([q_p, a_p], axis=1)
    score = qa @ lin_w.T + lin_b
    m = score.max(axis=1, keepdims=True)
    ls = score - m
    lse = np.log(np.exp(ls).sum(axis=1, keepdims=True))
    return (ls - lse).astype(np.float32)


# revision 13
# speedup vs baseline: 4.5479x; 1.6349x over previous
"""Trainium2 Bass kernel for NnqlmCnnBasedLstm.

Math (per batch item, per input sequence q/a):
  xe = embed[idx]                      (L, D)       D = 128
  dens_t = outer(xe_t, xe_t)/(|xe_t|^2 + 1e-4)     (D, D), symmetric
  2-layer ConvLSTM over L=40 steps; each gate g:
    pre_g = conv2d([xt; h], W_g, stride=(2,1), pad=(1,1)) + b_g  on (2D, D) -> (D, D)
  c = sig(f)*c + ig*tanh(cc); h = og*tanh(c)
  out = max_t h2_t  -> flatten -> concat(q,a) -> linear(2) -> log_softmax

Device strategy (8 cores, data parallel over B=32 -> 4 items/core, each with a
q-chain and an a-chain = 8 chains/core):
  * State kept TRANSPOSED: tiles are (w partitions, j free).  Densities are
    symmetric, precomputed on HOST, and DMAed per step (DMA engines are idle).
  * conv: out_T[w, j] = sum_{dh,dw} W[dh,dw] * inp_T[w-1+dw, 2j-1+dh].
    For each dh this is a 3-diagonal Toeplitz band matrix (over w) applied via
    the TensorEngine (bf16 stationary+moving = 1 cycle/row, half-cost
    ldweights), with the (2j-1+dh) selection expressed as a stride-2 free-axis
    access pattern on the moving operand.  4 gates x 4 dh matmuls accumulate
    in fp32 PSUM; all 8 chains batched in the moving free dim.
  * The two ConvLSTM layers are software-pipelined with a 1-step skew
    (emit P0(t) then P1(t-1)): the PE runs one layer's matmul burst while the
    other layer's Activation/Vector chain completes, so the PE never stalls
    and stays at its top p-state.
  * sigmoid/tanh (+conv bias) on ScalarE reading PSUM; cell updates on
    VectorE in fp32; h writes round to bf16; h1 fan-out copy on Pool.
  * Embedding gather, final linear + log_softmax on host (tiny).
"""

import os
import sys

import numpy as np

for _p in ("/opt/trn_rl_repo", "/root/.axon_site/_ro/trn_rl_repo"):
    if os.path.isdir(_p) and _p not in sys.path:
        sys.path.insert(0, _p)

B, L, D, V, NL = 32, 40, 128, 32000, 2
NCORES = 8
CH = 8            # chains per core: 4 batch items x {q, a}
SEG = 2 * D + 2   # per-chain column span in the input tile: [0]=0, [1..128]=x, [129..256]=h, [257]=0
NF = CH * SEG

_CACHE = {}


def _build_nc(L=L):
    import concourse.bass as bass
    import concourse.bacc as bacc
    import concourse.mybir as mybir
    from concourse import tile

    f32 = mybir.dt.float32
    bf16 = mybir.dt.bfloat16
    AF = mybir.ActivationFunctionType
    ALU = mybir.AluOpType

    nc = bacc.Bacc(None, target_bir_lowering=False)

    dens_d = nc.dram_tensor("dens", (L, D, CH * D), bf16, kind="ExternalInput")
    st_d = nc.dram_tensor("st", (NL * 4 * 4, D, D), bf16, kind="ExternalInput")
    bias_d = nc.dram_tensor("bias", (D, NL * 4), f32, kind="ExternalInput")
    out_d = nc.dram_tensor("mp_out", (D, CH * D), bf16, kind="ExternalOutput")

    with tile.TileContext(nc) as tc:
        with (
            tc.tile_pool(name="const", bufs=1) as constp,
            tc.tile_pool(name="state", bufs=1) as statep,
            tc.tile_pool(name="gate", bufs=2) as gatep,
            tc.tile_pool(name="psum", bufs=1, space="PSUM") as psump,
        ):
            # ---- constants ----
            stT = constp.tile([D, NL * 4 * 4 * D], bf16, tag="stT")
            for i in range(NL * 4 * 4):
                nc.sync.dma_start(stT[:, i * D:(i + 1) * D], st_d[i])

            bias = constp.tile([D, NL * 4], f32, tag="bias")
            nc.sync.dma_start(bias[:], bias_d[:])

            # ---- persistent state ----
            c_l = [statep.tile([D, CH * D], bf16, tag=f"c{l}", name=f"c{l}") for l in range(NL)]
            mp = statep.tile([D, CH * D], bf16, tag="mp")
            for l in range(NL):
                nc.vector.memset(c_l[l][:], 0.0)
            nc.vector.memset(mp[:], -1e30)

            def seg3(t):  # (p, s, c) view of an input tile
                return t[:].rearrange("p (s c) -> p s c", s=CH)

            def seg4(t):  # (p, s, c2, two) parity view for stride-2 j access
                return t[:].rearrange("p (s c two) -> p s c two", s=CH, two=2)

            # ping-pong input tiles per layer: I0[t%2] = layer-1 input at t,
            # I1[t%2] = layer-2 input at t.
            I0 = [statep.tile([D, NF], bf16, tag=f"I0{p}", name=f"I0{p}") for p in range(2)]
            I1 = [statep.tile([D, NF], bf16, tag=f"I1{p}", name=f"I1{p}") for p in range(2)]
            for t_ in I0 + I1:
                v = seg3(t_)
                nc.gpsimd.memset(v[:, :, 0:1], 0.0)
                nc.gpsimd.memset(v[:, :, SEG - 1:SEG], 0.0)
            nc.gpsimd.memset(seg3(I0[0])[:, :, 129:129 + D], 0.0)  # h1_{-1} = 0
            nc.gpsimd.memset(seg3(I1[0])[:, :, 129:129 + D], 0.0)  # h2_{-1} = 0

            # density prefetch for steps 0 and 1
            nc.sync.dma_start(seg3(I0[0])[:, :, 1:1 + D], dens_d[0])
            nc.sync.dma_start(seg3(I0[1])[:, :, 1:1 + D], dens_d[1])

            GTAG = ["pf", "pi", "po", "pc"]

            def emit_P(l, t):
                """One layer-step: matmul burst -> activations -> cell update
                -> route h to consumers."""
                inp = I0[t % 2] if l == 0 else I1[t % 2]
                i4 = seg4(inp)
                ps = [psump.tile([D, CH * D], f32, tag=GTAG[g], name=GTAG[g])
                      for g in range(4)]
                for g in range(4):
                    for half in range(2):
                        for dh in range(4):
                            idx = (l * 4 + g) * 4 + dh
                            rhs = i4[:, half * 4:(half + 1) * 4,
                                     dh // 2: dh // 2 + D, dh % 2]
                            nc.tensor.matmul(
                                ps[g][:, half * 512:(half + 1) * 512],
                                stT[:, idx * D:(idx + 1) * D],
                                rhs,
                                start=(dh == 0), stop=(dh == 3),
                            )
                # --- activations (bias folded in) ---
                fg = gatep.tile([D, CH * D], bf16, tag="fg")
                ig = gatep.tile([D, CH * D], bf16, tag="ig")
                og = gatep.tile([D, CH * D], bf16, tag="og")
                cs = gatep.tile([D, CH * D], bf16, tag="cs")
                for g, dst in enumerate((fg, ig, og)):
                    nc.scalar.activation(dst[:], ps[g][:], AF.Sigmoid,
                                         bias=bias[:, l * 4 + g: l * 4 + g + 1])
                nc.scalar.activation(cs[:], ps[3][:], AF.Tanh,
                                     bias=bias[:, l * 4 + 3: l * 4 + 4])
                # --- cell update (all bf16 in SBUF: DVE 4x mode) ---
                t1 = gatep.tile([D, CH * D], bf16, tag="t1")
                t2 = gatep.tile([D, CH * D], bf16, tag="t2")
                nc.vector.tensor_mul(t1[:], fg[:], c_l[l][:])
                nc.vector.tensor_mul(t2[:], ig[:], cs[:])
                nc.vector.tensor_add(c_l[l][:], t1[:], t2[:])
                th = gatep.tile([D, CH * D], bf16, tag="th")
                nc.scalar.activation(th[:], c_l[l][:], AF.Tanh)
                # --- h = og * tanh(c): route to consumers ---
                if l == 0:
                    # h1_t -> x-part of layer-2 input (this step)
                    h1dst = seg3(I1[t % 2])[:, :, 1:1 + D]
                    nc.vector.tensor_mul(h1dst, og[:], th[:])
                    if t + 1 < L:
                        # h1_t -> h-part of next layer-1 input
                        nc.vector.tensor_mul(
                            seg3(I0[(t + 1) % 2])[:, :, 129:129 + D],
                            og[:], th[:])
                else:
                    if t + 1 < L:
                        h2dst = seg3(I1[(t + 1) % 2])[:, :, 129:129 + D]
                        nc.vector.tensor_mul(h2dst, og[:], th[:])
                        nc.vector.tensor_tensor(mp[:], mp[:], h2dst, op=ALU.max)
                    else:
                        h2 = gatep.tile([D, CH * D], bf16, tag="h2last")
                        nc.vector.tensor_mul(h2[:], og[:], th[:])
                        nc.vector.tensor_tensor(mp[:], mp[:], h2[:], op=ALU.max)

            # skewed schedule: PE alternates between the two layers' bursts
            emit_P(0, 0)
            for t in range(1, L):
                if t + 1 < L:
                    nc.sync.dma_start(seg3(I0[(t + 1) % 2])[:, :, 1:1 + D],
                                      dens_d[t + 1])
                emit_P(0, t)
                emit_P(1, t - 1)
            emit_P(1, L - 1)

            nc.sync.dma_start(out_d[:], mp[:])

    nc.compile()
    return nc


def _prep_core_inputs(dens_all, st, bias_arr, core):
    """dens_all: (B, 2, L, D, D) bf16 densities (axis1: 0=q, 1=a)."""
    import ml_dtypes
    sl = slice(4 * core, 4 * core + 4)
    ch = np.concatenate([dens_all[sl, 0], dens_all[sl, 1]], axis=0)  # (8, L, D, D)
    # dens[t, w, s*128+j] = ch[s, t, w, j]
    dens = np.ascontiguousarray(ch.transpose(1, 2, 0, 3)).reshape(L, D, CH * D)
    return {"dens": dens, "st": st, "bias": bias_arr}


def kernel(q, a, embed, conv_w, conv_b, lin_w, lin_b):
    import ml_dtypes
    from concourse import bass_utils

    bf16 = ml_dtypes.bfloat16
    q = np.asarray(q); a = np.asarray(a)
    embed = np.asarray(embed, np.float32)
    conv_w = np.asarray(conv_w, np.float32)
    conv_b = np.asarray(conv_b, np.float32)
    lin_w = np.asarray(lin_w, np.float32)
    lin_b = np.asarray(lin_b, np.float32)

    # host: embedding gather + density (normalized outer products)
    idx = np.stack([q, a], axis=1).astype(np.int64)            # (B, 2, L)
    xe = embed[idx].astype(np.float64)                         # (B, 2, L, D)
    dot = np.sum(xe * xe, axis=-1, keepdims=True) + 1e-4
    xe_y = (xe / np.sqrt(dot)).astype(np.float32)
    dens_all = np.einsum('bslw,bslj->bslwj', xe_y, xe_y).astype(bf16)

    # host: Toeplitz band stationaries  lhsT[(l,g,dh)] = B^T,
    # B[w, w'] = W[dh, w'-w+1]  (3 diagonals)
    st = np.zeros((NL * 4 * 4, D, D), np.float32)
    for l in range(NL):
        for g in range(4):
            W = conv_w[l, g, 0, 0]                             # (4, 3)
            for dh in range(4):
                Bm = sum(W[dh, dw] * np.eye(D, k=dw - 1) for dw in range(3))
                st[(l * 4 + g) * 4 + dh] = Bm.T.astype(np.float32)
    st = st.astype(bf16)
    bias_arr = np.tile(conv_b.reshape(1, -1), (D, 1)).astype(np.float32)

    if "nc" not in _CACHE:
        _CACHE["nc"] = _build_nc()
    nc = _CACHE["nc"]

    in_maps = [_prep_core_inputs(dens_all, st, bias_arr, i) for i in range(NCORES)]
    _CACHE["in_maps"] = in_maps
    res = bass_utils.run_bass_kernel_spmd(nc, in_maps, core_ids=list(range(NCORES)))

    # host: unshard + final linear + log_softmax
    q_p = np.zeros((B, D * D), np.float32)
    a_p = np.zeros((B, D * D), np.float32)
    for i in range(NCORES):
        out = np.asarray(res.results[i]["mp_out"]).astype(np.float32)  # (D w, CH*D)
        for s in range(CH):
            mp_T = out[:, s * D:(s + 1) * D]                   # (w, j)
            flat = np.ascontiguousarray(mp_T.T).reshape(-1)    # j-major
            if s < 4:
                q_p[4 * i + s] = flat
            else:
                a_p[4 * i + s - 4] = flat
    qa = np.concatenate([q_p, a_p], axis=1)
    score = qa @ lin_w.T + lin_b
    m = score.max(axis=1, keepdims=True)
    ls = score - m
    lse = np.log(np.exp(ls).sum(axis=1, keepdims=True))
    return (ls - lse).astype(np.float32)


# revision 14
# speedup vs baseline: 5.3597x; 1.1785x over previous
"""Trainium2 Bass kernel for NnqlmCnnBasedLstm.

Math (per batch item, per input sequence q/a):
  xe = embed[idx]                      (L, D)       D = 128
  dens_t = outer(xe_t, xe_t)/(|xe_t|^2 + 1e-4)     (D, D), symmetric
  2-layer ConvLSTM over L=40 steps; each gate g:
    pre_g = conv2d([xt; h], W_g, stride=(2,1), pad=(1,1)) + b_g  on (2D, D) -> (D, D)
  c = sig(f)*c + ig*tanh(cc); h = og*tanh(c)
  out = max_t h2_t  -> flatten -> concat(q,a) -> linear(2) -> log_softmax

Device strategy (8 cores, data parallel over B=32 -> 4 items/core, each with a
q-chain and an a-chain = 8 chains/core):
  * State kept TRANSPOSED: tiles are (w partitions, j free).  Densities are
    symmetric, precomputed on HOST, and DMAed per step (DMA engines are idle).
  * conv: out_T[w, j] = sum_{dh,dw} W[dh,dw] * inp_T[w-1+dw, 2j-1+dh].
    For each dh this is a 3-diagonal Toeplitz band matrix (over w) applied on
    the TensorEngine.  fp8e4m3 + perf_mode=DoubleRow packs the (dh0,dh1) and
    (dh2,dh3) band pairs as two K-planes of one matmul (the plane pairs are
    adjacent columns of the input tile), halving the matmul count; fp32 PSUM
    accumulates the two pair-matmuls per gate.  All 8 chains batched in the
    moving free dim (2 x 512-col halves per PSUM bank limit).
  * The two ConvLSTM layers are software-pipelined with a 1-step skew
    (emit P0(t) then P1(t-1)): the PE runs one layer's matmul burst while the
    other layer's Activation/Vector chain completes.
  * sigmoid/tanh (+conv bias) on ScalarE reading PSUM -> bf16 gates; cell
    updates on VectorE in bf16 (4x perf mode); h rounds to fp8 into the next
    input tiles; a bf16 copy of h2 feeds the running max.
  * Embedding gather, final linear + log_softmax on host (tiny).
"""

import os
import sys

import numpy as np

for _p in ("/opt/trn_rl_repo", "/root/.axon_site/_ro/trn_rl_repo"):
    if os.path.isdir(_p) and _p not in sys.path:
        sys.path.insert(0, _p)

B, L, D, V, NL = 32, 40, 128, 32000, 2
NCORES = 8
CH = 8            # chains per core: 4 batch items x {q, a}
SEG = 2 * D + 2   # per-chain column span in the input tile: [0]=0, [1..128]=x, [129..256]=h, [257]=0
NF = CH * SEG

_CACHE = {}


def _build_nc(L=L):
    import concourse.bass as bass
    import concourse.bacc as bacc
    import concourse.mybir as mybir
    from concourse import tile

    f32 = mybir.dt.float32
    bf16 = mybir.dt.bfloat16
    fp8 = mybir.dt.float8e4
    i8 = mybir.dt.int8
    AF = mybir.ActivationFunctionType
    ALU = mybir.AluOpType
    DR = mybir.MatmulPerfMode.DoubleRow

    nc = bacc.Bacc(None, target_bir_lowering=False)

    dens_d = nc.dram_tensor("dens", (L, D, CH * D), fp8, kind="ExternalInput")
    st_d = nc.dram_tensor("st", (NL * 4 * 2, D, 2 * D), fp8, kind="ExternalInput")
    bias_d = nc.dram_tensor("bias", (D, NL * 4), f32, kind="ExternalInput")
    out_d = nc.dram_tensor("mp_out", (D, CH * D), bf16, kind="ExternalOutput")

    with tile.TileContext(nc) as tc:
        with (
            tc.tile_pool(name="const", bufs=1) as constp,
            tc.tile_pool(name="state", bufs=1) as statep,
            tc.tile_pool(name="gate", bufs=2) as gatep,
            tc.tile_pool(name="psum", bufs=1, space="PSUM") as psump,
        ):
            # ---- constants ----
            # stT[(l,g,pair)]: [K=128, plane(2) x M(128)] interleaved band pair
            stT = constp.tile([D, NL * 4 * 2 * 2 * D], fp8, tag="stT")
            bias = constp.tile([D, NL * 4], f32, tag="bias")

            # ---- persistent state ----
            c_l = [statep.tile([D, CH * D], bf16, tag=f"c{l}", name=f"c{l}") for l in range(NL)]
            mp = statep.tile([D, CH * D], bf16, tag="mp")

            I0 = [statep.tile([D, NF], fp8, tag=f"I0{p}", name=f"I0{p}") for p in range(2)]
            I1 = [statep.tile([D, NF], fp8, tag=f"I1{p}", name=f"I1{p}") for p in range(2)]

            def seg3(t):  # (p, s, c) view of an input tile
                return t[:].rearrange("p (s c) -> p s c", s=CH)

            def pairview(t):  # (p, two, s, j129) DoubleRow moving view
                # col = s*258 + j*2 + two;  plane dim (two) must be free dim 1
                return t[:].rearrange("p (s j two) -> p two s j", s=CH, two=2)

            # startup: density for step 0/1 first, then constants
            nc.sync.dma_start(seg3(I0[0])[:, :, 1:1 + D], dens_d[0])
            nc.sync.dma_start(bias[:], bias_d[:])
            for i in range(NL * 4 * 2):
                nc.scalar.dma_start(stT[:, i * 2 * D:(i + 1) * 2 * D], st_d[i])
            nc.sync.dma_start(seg3(I0[1])[:, :, 1:1 + D], dens_d[1])

            for l in range(NL):
                nc.vector.memset(c_l[l][:], 0.0)
            nc.vector.memset(mp[:], -1e30)
            for t_ in I0 + I1:
                v = seg3(t_)
                nc.gpsimd.memset(v[:, :, 0:1].bitcast(i8), 0)
                nc.gpsimd.memset(v[:, :, SEG - 1:SEG].bitcast(i8), 0)
            nc.gpsimd.memset(seg3(I0[0])[:, :, 129:129 + D].bitcast(i8), 0)  # h1_{-1}
            nc.gpsimd.memset(seg3(I1[0])[:, :, 129:129 + D].bitcast(i8), 0)  # h2_{-1}

            GTAG = ["pf", "pi", "po", "pc"]

            def emit_P(l, t):
                """One layer-step: matmul burst -> activations -> cell update
                -> route h to consumers."""
                inp = I0[t % 2] if l == 0 else I1[t % 2]
                pv = pairview(inp)
                ps = [psump.tile([D, CH * D], f32, tag=GTAG[g], name=GTAG[g])
                      for g in range(4)]
                for g in range(4):
                    for half in range(2):
                        for pr in range(2):  # dh pairs (0,1) and (2,3)
                            idx = (l * 4 + g) * 2 + pr
                            lhsT = stT[:, idx * 2 * D:(idx + 1) * 2 * D] \
                                .rearrange("p (two m) -> p two m", two=2)
                            rhs = pv[:, :, half * 4:(half + 1) * 4, pr:pr + D]
                            nc.tensor.matmul(
                                ps[g][:, half * 512:(half + 1) * 512],
                                lhsT, rhs,
                                start=(pr == 0), stop=(pr == 1),
                                perf_mode=DR,
                            )
                # --- activations (bias folded in) ---
                fg = gatep.tile([D, CH * D], bf16, tag="fg")
                ig = gatep.tile([D, CH * D], bf16, tag="ig")
                og = gatep.tile([D, CH * D], bf16, tag="og")
                cs = gatep.tile([D, CH * D], bf16, tag="cs")
                for g, dst in enumerate((fg, ig, og)):
                    nc.scalar.activation(dst[:], ps[g][:], AF.Sigmoid,
                                         bias=bias[:, l * 4 + g: l * 4 + g + 1])
                nc.scalar.activation(cs[:], ps[3][:], AF.Tanh,
                                     bias=bias[:, l * 4 + 3: l * 4 + 4])
                # --- cell update (all bf16 in SBUF: DVE 4x mode) ---
                t1 = gatep.tile([D, CH * D], bf16, tag="t1")
                t2 = gatep.tile([D, CH * D], bf16, tag="t2")
                nc.vector.tensor_mul(t1[:], fg[:], c_l[l][:])
                nc.vector.tensor_mul(t2[:], ig[:], cs[:])
                nc.vector.tensor_add(c_l[l][:], t1[:], t2[:])
                th = gatep.tile([D, CH * D], bf16, tag="th")
                nc.scalar.activation(th[:], c_l[l][:], AF.Tanh)
                # --- h = og * tanh(c): route to consumers (rounds to fp8) ---
                if l == 0:
                    nc.vector.tensor_mul(seg3(I1[t % 2])[:, :, 1:1 + D],
                                         og[:], th[:])
                    if t + 1 < L:
                        nc.vector.tensor_mul(
                            seg3(I0[(t + 1) % 2])[:, :, 129:129 + D],
                            og[:], th[:])
                else:
                    # bf16 copy for the running max (pooling stays bf16)
                    h2 = gatep.tile([D, CH * D], bf16, tag="h2")
                    nc.vector.tensor_mul(h2[:], og[:], th[:])
                    nc.vector.tensor_tensor(mp[:], mp[:], h2[:], op=ALU.max)
                    if t + 1 < L:
                        nc.vector.tensor_mul(
                            seg3(I1[(t + 1) % 2])[:, :, 129:129 + D],
                            og[:], th[:])

            # skewed schedule: PE alternates between the two layers' bursts
            emit_P(0, 0)
            for t in range(1, L):
                if t + 1 < L:
                    nc.sync.dma_start(seg3(I0[(t + 1) % 2])[:, :, 1:1 + D],
                                      dens_d[t + 1])
                emit_P(0, t)
                emit_P(1, t - 1)
            emit_P(1, L - 1)

            nc.sync.dma_start(out_d[:], mp[:])

    nc.compile()
    return nc


def _prep_core_inputs(dens_all, st, bias_arr, core):
    """dens_all: (B, 2, L, D, D) fp8 densities (axis1: 0=q, 1=a)."""
    sl = slice(4 * core, 4 * core + 4)
    ch = np.concatenate([dens_all[sl, 0], dens_all[sl, 1]], axis=0)  # (8, L, D, D)
    # dens[t, w, s*128+j] = ch[s, t, w, j]
    dens = np.ascontiguousarray(ch.transpose(1, 2, 0, 3)).reshape(L, D, CH * D)
    return {"dens": dens, "st": st, "bias": bias_arr}


def kernel(q, a, embed, conv_w, conv_b, lin_w, lin_b):
    import ml_dtypes
    from concourse import bass_utils

    fp8 = ml_dtypes.float8_e4m3
    q = np.asarray(q); a = np.asarray(a)
    embed = np.asarray(embed, np.float32)
    conv_w = np.asarray(conv_w, np.float32)
    conv_b = np.asarray(conv_b, np.float32)
    lin_w = np.asarray(lin_w, np.float32)
    lin_b = np.asarray(lin_b, np.float32)

    # host: embedding gather + density (normalized outer products)
    idx = np.stack([q, a], axis=1).astype(np.int64)            # (B, 2, L)
    xe = embed[idx].astype(np.float64)                         # (B, 2, L, D)
    dot = np.sum(xe * xe, axis=-1, keepdims=True) + 1e-4
    xe_y = (xe / np.sqrt(dot)).astype(np.float32)
    dens_all = np.einsum('bslw,bslj->bslwj', xe_y, xe_y).astype(fp8)

    # host: Toeplitz band stationaries, DoubleRow pair-interleaved:
    # st[(l,g,pair), k, plane*128 + m] = B_{2*pair+plane}^T[k, m],
    # B_dh[w, w'] = W[dh, w'-w+1]  (3 diagonals)
    st = np.zeros((NL * 4 * 2, D, 2 * D), np.float32)
    for l in range(NL):
        for g in range(4):
            W = conv_w[l, g, 0, 0]                             # (4, 3)
            for dh in range(4):
                Bm = sum(W[dh, dw] * np.eye(D, k=dw - 1) for dw in range(3))
                pr, pp = dh // 2, dh % 2
                st[(l * 4 + g) * 2 + pr, :, pp * D:(pp + 1) * D] = \
                    Bm.T.astype(np.float32)
    st = st.astype(fp8)
    bias_arr = np.tile(conv_b.reshape(1, -1), (D, 1)).astype(np.float32)

    if "nc" not in _CACHE:
        _CACHE["nc"] = _build_nc()
    nc = _CACHE["nc"]

    in_maps = [_prep_core_inputs(dens_all, st, bias_arr, i) for i in range(NCORES)]
    _CACHE["in_maps"] = in_maps
    res = bass_utils.run_bass_kernel_spmd(nc, in_maps, core_ids=list(range(NCORES)))

    # host: unshard + final linear + log_softmax
    q_p = np.zeros((B, D * D), np.float32)
    a_p = np.zeros((B, D * D), np.float32)
    for i in range(NCORES):
        out = np.asarray(res.results[i]["mp_out"]).astype(np.float32)  # (D w, CH*D)
        for s in range(CH):
            mp_T = out[:, s * D:(s + 1) * D]                   # (w, j)
            flat = np.ascontiguousarray(mp_T.T).reshape(-1)    # j-major
            if s < 4:
                q_p[4 * i + s] = flat
            else:
                a_p[4 * i + s - 4] = flat
    qa = np.concatenate([q_p, a_p], axis=1)
    score = qa @ lin_w.T + lin_b
    m = score.max(axis=1, keepdims=True)
    ls = score - m
    lse = np.log(np.exp(ls).sum(axis=1, keepdims=True))
    return (ls - lse).astype(np.float32)


# revision 16
# speedup vs baseline: 5.7092x; 1.0652x over previous
"""Trainium2 Bass kernel for NnqlmCnnBasedLstm.

Math (per batch item, per input sequence q/a):
  xe = embed[idx]                      (L, D)       D = 128
  dens_t = outer(xe_t, xe_t)/(|xe_t|^2 + 1e-4)     (D, D), symmetric
  2-layer ConvLSTM over L=40 steps; each gate g:
    pre_g = conv2d([xt; h], W_g, stride=(2,1), pad=(1,1)) + b_g  on (2D, D) -> (D, D)
  c = sig(f)*c + ig*tanh(cc); h = og*tanh(c)
  out = max_t h2_t  -> flatten -> concat(q,a) -> linear(2) -> log_softmax

Device strategy (8 cores, data parallel over B=32 -> 4 items/core, each with a
q-chain and an a-chain = 8 chains/core):
  * State kept TRANSPOSED: tiles are (w partitions, j free).  Densities are
    symmetric, precomputed on HOST, and DMAed per step (DMA engines are idle).
  * conv: out_T[w, j] = sum_{dh,dw} W[dh,dw] * inp_T[w-1+dw, 2j-1+dh].
    For each dh this is a 3-diagonal Toeplitz band matrix (over w) applied on
    the TensorEngine.  fp8e4m3 + perf_mode=DoubleRow packs the (dh0,dh1) and
    (dh2,dh3) band pairs as two K-planes of one matmul (the plane pairs are
    adjacent columns of the input tile), halving the matmul count; fp32 PSUM
    accumulates the two pair-matmuls per gate.  All 8 chains batched in the
    moving free dim (2 x 512-col halves per PSUM bank limit).
  * The two ConvLSTM layers are software-pipelined with a 1-step skew
    (emit P0(t) then P1(t-1)): the PE runs one layer's matmul burst while the
    other layer's Activation/Vector chain completes.
  * sigmoid/tanh (+conv bias) on ScalarE reading PSUM -> bf16 gates; cell
    updates on VectorE in bf16 (4x perf mode); h rounds to fp8 into the next
    input tiles; a bf16 copy of h2 feeds the running max.
  * Embedding gather, final linear + log_softmax on host (tiny).
"""

import os
import sys

import numpy as np

for _p in ("/opt/trn_rl_repo", "/root/.axon_site/_ro/trn_rl_repo"):
    if os.path.isdir(_p) and _p not in sys.path:
        sys.path.insert(0, _p)

B, L, D, V, NL = 32, 40, 128, 32000, 2
NCORES = 8
CH = 8            # chains per core: 4 batch items x {q, a}
SEG = 2 * D + 2   # per-chain column span in the input tile: [0]=0, [1..128]=x, [129..256]=h, [257]=0
NF = CH * SEG

_CACHE = {}


def _build_nc(L=L):
    import concourse.bass as bass
    import concourse.bacc as bacc
    import concourse.mybir as mybir
    from concourse import tile

    f32 = mybir.dt.float32
    bf16 = mybir.dt.bfloat16
    fp8 = mybir.dt.float8e4
    i8 = mybir.dt.int8
    AF = mybir.ActivationFunctionType
    ALU = mybir.AluOpType
    DR = mybir.MatmulPerfMode.DoubleRow

    nc = bacc.Bacc(None, target_bir_lowering=False)

    dens_d = nc.dram_tensor("dens", (L, D, CH * D), fp8, kind="ExternalInput")
    st_d = nc.dram_tensor("st", (NL * 4 * 2, D, 2 * D), fp8, kind="ExternalInput")
    bias_d = nc.dram_tensor("bias", (D, NL * 4), f32, kind="ExternalInput")
    out_d = nc.dram_tensor("mp_out", (D, CH * D), bf16, kind="ExternalOutput")

    with tile.TileContext(nc) as tc:
        with (
            tc.tile_pool(name="const", bufs=1) as constp,
            tc.tile_pool(name="state", bufs=1) as statep,
            tc.tile_pool(name="gate", bufs=2) as gatep,
            tc.tile_pool(name="psum", bufs=1, space="PSUM") as psump,
        ):
            # ---- constants ----
            # stT[(l,g,pair)]: [K=128, plane(2) x M(128)] interleaved band pair
            stT = constp.tile([D, NL * 4 * 2 * 2 * D], fp8, tag="stT")
            bias = constp.tile([D, NL * 4], f32, tag="bias")

            # ---- persistent state ----
            c_l = [statep.tile([D, CH * D], bf16, tag=f"c{l}", name=f"c{l}") for l in range(NL)]
            mp = statep.tile([D, CH * D], bf16, tag="mp")

            I0 = [statep.tile([D, NF], fp8, tag=f"I0{p}", name=f"I0{p}") for p in range(2)]
            I1 = [statep.tile([D, NF], fp8, tag=f"I1{p}", name=f"I1{p}") for p in range(2)]

            def seg3(t):  # (p, s, c) view of an input tile
                return t[:].rearrange("p (s c) -> p s c", s=CH)

            def pairview(t):  # (p, two, s, j129) DoubleRow moving view
                # col = s*258 + j*2 + two;  plane dim (two) must be free dim 1
                return t[:].rearrange("p (s j two) -> p two s j", s=CH, two=2)

            # startup: density for step 0/1 first, then constants
            nc.sync.dma_start(seg3(I0[0])[:, :, 1:1 + D], dens_d[0])
            nc.sync.dma_start(bias[:], bias_d[:])
            for i in range(NL * 4 * 2):
                nc.scalar.dma_start(stT[:, i * 2 * D:(i + 1) * 2 * D], st_d[i])
            nc.sync.dma_start(seg3(I0[1])[:, :, 1:1 + D], dens_d[1])

            for l in range(NL):
                nc.vector.memset(c_l[l][:], 0.0)
            nc.vector.memset(mp[:], -1e30)
            for t_ in I0 + I1:
                v = seg3(t_)
                nc.gpsimd.memset(v[:, :, 0:1].bitcast(i8), 0)
                nc.gpsimd.memset(v[:, :, SEG - 1:SEG].bitcast(i8), 0)
            nc.gpsimd.memset(seg3(I0[0])[:, :, 129:129 + D].bitcast(i8), 0)  # h1_{-1}
            nc.gpsimd.memset(seg3(I1[0])[:, :, 129:129 + D].bitcast(i8), 0)  # h2_{-1}

            GTAG = ["pf", "pi", "po", "pc"]

            def emit_P(l, t):
                """One layer-step: matmul burst -> activations -> cell update
                -> route h to consumers."""
                inp = I0[t % 2] if l == 0 else I1[t % 2]
                pv = pairview(inp)
                ps = {g: psump.tile([D, CH * D], f32, tag=GTAG[g], name=GTAG[g])
                      for g in range(4)}
                # c-gate first: its activation is the head of the dependency
                # chain (cs -> t2 -> c -> th -> h)
                for g in (3, 0, 1, 2):
                    for half in range(2):
                        for pr in range(2):  # dh pairs (0,1) and (2,3)
                            idx = (l * 4 + g) * 2 + pr
                            lhsT = stT[:, idx * 2 * D:(idx + 1) * 2 * D] \
                                .rearrange("p (two m) -> p two m", two=2)
                            rhs = pv[:, :, half * 4:(half + 1) * 4, pr:pr + D]
                            nc.tensor.matmul(
                                ps[g][:, half * 512:(half + 1) * 512],
                                lhsT, rhs,
                                start=(pr == 0), stop=(pr == 1),
                                perf_mode=DR,
                            )
                # --- activations (bias folded in), chain-critical order ---
                fg = gatep.tile([D, CH * D], bf16, tag="fg")
                ig = gatep.tile([D, CH * D], bf16, tag="ig")
                og = gatep.tile([D, CH * D], bf16, tag="og")
                cs = gatep.tile([D, CH * D], bf16, tag="cs")
                t1 = gatep.tile([D, CH * D], bf16, tag="t1")
                t2 = gatep.tile([D, CH * D], bf16, tag="t2")
                th = gatep.tile([D, CH * D], bf16, tag="th")
                nc.scalar.activation(cs[:], ps[3][:], AF.Tanh,
                                     bias=bias[:, l * 4 + 3: l * 4 + 4])
                nc.scalar.activation(fg[:], ps[0][:], AF.Sigmoid,
                                     bias=bias[:, l * 4 + 0: l * 4 + 1])
                nc.scalar.activation(ig[:], ps[1][:], AF.Sigmoid,
                                     bias=bias[:, l * 4 + 1: l * 4 + 2])
                # --- cell update (all bf16 in SBUF: DVE 4x mode) ---
                nc.vector.tensor_mul(t1[:], fg[:], c_l[l][:])
                nc.vector.tensor_mul(t2[:], ig[:], cs[:])
                nc.vector.tensor_add(c_l[l][:], t1[:], t2[:])
                nc.scalar.activation(th[:], c_l[l][:], AF.Tanh)
                nc.scalar.activation(og[:], ps[2][:], AF.Sigmoid,
                                     bias=bias[:, l * 4 + 2: l * 4 + 3])
                # --- h = og * tanh(c): route to consumers (rounds to fp8) ---
                if l == 0:
                    if t + 1 < L:
                        # gates the very next P0 burst: emit first
                        nc.vector.tensor_mul(
                            seg3(I0[(t + 1) % 2])[:, :, 129:129 + D],
                            og[:], th[:])
                    nc.vector.tensor_mul(seg3(I1[t % 2])[:, :, 1:1 + D],
                                         og[:], th[:])
                else:
                    # bf16 copy for the running max (pooling stays bf16)
                    h2 = gatep.tile([D, CH * D], bf16, tag="h2")
                    nc.vector.tensor_mul(h2[:], og[:], th[:])
                    nc.vector.tensor_tensor(mp[:], mp[:], h2[:], op=ALU.max)
                    if t + 1 < L:
                        nc.vector.tensor_mul(
                            seg3(I1[(t + 1) % 2])[:, :, 129:129 + D],
                            og[:], th[:])

            # skewed schedule: PE alternates between the two layers' bursts
            emit_P(0, 0)
            for t in range(1, L):
                if t + 1 < L:
                    nc.sync.dma_start(seg3(I0[(t + 1) % 2])[:, :, 1:1 + D],
                                      dens_d[t + 1])
                emit_P(0, t)
                emit_P(1, t - 1)
            emit_P(1, L - 1)

            nc.sync.dma_start(out_d[:], mp[:])

    nc.compile()
    return nc


def _prep_core_inputs(dens_all, st, bias_arr, core):
    """dens_all: (B, 2, L, D, D) fp8 densities (axis1: 0=q, 1=a)."""
    sl = slice(4 * core, 4 * core + 4)
    ch = np.concatenate([dens_all[sl, 0], dens_all[sl, 1]], axis=0)  # (8, L, D, D)
    # dens[t, w, s*128+j] = ch[s, t, w, j]
    dens = np.ascontiguousarray(ch.transpose(1, 2, 0, 3)).reshape(L, D, CH * D)
    return {"dens": dens, "st": st, "bias": bias_arr}


def kernel(q, a, embed, conv_w, conv_b, lin_w, lin_b):
    import ml_dtypes
    from concourse import bass_utils

    fp8 = ml_dtypes.float8_e4m3
    q = np.asarray(q); a = np.asarray(a)
    embed = np.asarray(embed, np.float32)
    conv_w = np.asarray(conv_w, np.float32)
    conv_b = np.asarray(conv_b, np.float32)
    lin_w = np.asarray(lin_w, np.float32)
    lin_b = np.asarray(lin_b, np.float32)

    # host: embedding gather + density (normalized outer products)
    idx = np.stack([q, a], axis=1).astype(np.int64)            # (B, 2, L)
    xe = embed[idx].astype(np.float64)                         # (B, 2, L, D)
    dot = np.sum(xe * xe, axis=-1, keepdims=True) + 1e-4
    xe_y = (xe / np.sqrt(dot)).astype(np.float32)
    dens_all = np.einsum('bslw,bslj->bslwj', xe_y, xe_y).astype(fp8)

    # host: Toeplitz band stationaries, DoubleRow pair-interleaved:
    # st[(l,g,pair), k, plane*128 + m] = B_{2*pair+plane}^T[k, m],
    # B_dh[w, w'] = W[dh, w'-w+1]  (3 diagonals)
    st = np.zeros((NL * 4 * 2, D, 2 * D), np.float32)
    for l in range(NL):
        for g in range(4):
            W = conv_w[l, g, 0, 0]                             # (4, 3)
            for dh in range(4):
                Bm = sum(W[dh, dw] * np.eye(D, k=dw - 1) for dw in range(3))
                pr, pp = dh // 2, dh % 2
                st[(l * 4 + g) * 2 + pr, :, pp * D:(pp + 1) * D] = \
                    Bm.T.astype(np.float32)
    st = st.astype(fp8)
    bias_arr = np.tile(conv_b.reshape(1, -1), (D, 1)).astype(np.float32)

    if "nc" not in _CACHE:
        _CACHE["nc"] = _build_nc()
    nc = _CACHE["nc"]

    in_maps = [_prep_core_inputs(dens_all, st, bias_arr, i) for i in range(NCORES)]
    _CACHE["in_maps"] = in_maps
    res = bass_utils.run_bass_kernel_spmd(nc, in_maps, core_ids=list(range(NCORES)))

    # host: unshard + final linear + log_softmax
    q_p = np.zeros((B, D * D), np.float32)
    a_p = np.zeros((B, D * D), np.float32)
    for i in range(NCORES):
        out = np.asarray(res.results[i]["mp_out"]).astype(np.float32)  # (D w, CH*D)
        for s in range(CH):
            mp_T = out[:, s * D:(s + 1) * D]                   # (w, j)
            flat = np.ascontiguousarray(mp_T.T).reshape(-1)    # j-major
            if s < 4:
                q_p[4 * i + s] = flat
            else:
                a_p[4 * i + s - 4] = flat
    qa = np.concatenate([q_p, a_p], axis=1)
    score = qa @ lin_w.T + lin_b
    m = score.max(axis=1, keepdims=True)
    ls = score - m
    lse = np.log(np.exp(ls).sum(axis=1, keepdims=True))
    return (ls - lse).astype(np.float32)


# revision 18
# speedup vs baseline: 5.7571x; 1.0084x over previous
"""Trainium2 Bass kernel for NnqlmCnnBasedLstm.

Math (per batch item, per input sequence q/a):
  xe = embed[idx]                      (L, D)       D = 128
  dens_t = outer(xe_t, xe_t)/(|xe_t|^2 + 1e-4)     (D, D), symmetric
  2-layer ConvLSTM over L=40 steps; each gate g:
    pre_g = conv2d([xt; h], W_g, stride=(2,1), pad=(1,1)) + b_g  on (2D, D) -> (D, D)
  c = sig(f)*c + ig*tanh(cc); h = og*tanh(c)
  out = max_t h2_t  -> flatten -> concat(q,a) -> linear(2) -> log_softmax

Device strategy (8 cores, data parallel over B=32 -> 4 items/core, each with a
q-chain and an a-chain = 8 chains/core):
  * State kept TRANSPOSED: tiles are (w partitions, j free).  Densities are
    symmetric, precomputed on HOST, and DMAed per step (DMA engines are idle).
  * conv: out_T[w, j] = sum_{dh,dw} W[dh,dw] * inp_T[w-1+dw, 2j-1+dh].
    For each dh this is a 3-diagonal Toeplitz band matrix (over w) applied on
    the TensorEngine.  fp8e4m3 + perf_mode=DoubleRow packs the (dh0,dh1) and
    (dh2,dh3) band pairs as two K-planes of one matmul (the plane pairs are
    adjacent columns of the input tile), halving the matmul count; fp32 PSUM
    accumulates the two pair-matmuls per gate.  All 8 chains batched in the
    moving free dim (2 x 512-col halves per PSUM bank limit).
  * The two ConvLSTM layers are software-pipelined with a 1-step skew
    (emit P0(t) then P1(t-1)): the PE runs one layer's matmul burst while the
    other layer's Activation/Vector chain completes.
  * sigmoid/tanh (+conv bias) on ScalarE reading PSUM -> bf16 gates; cell
    updates on VectorE in bf16 (4x perf mode); h rounds to fp8 into the next
    input tiles; a bf16 copy of h2 feeds the running max.
  * Embedding gather, final linear + log_softmax on host (tiny).
"""

import os
import sys

import numpy as np

for _p in ("/opt/trn_rl_repo", "/root/.axon_site/_ro/trn_rl_repo"):
    if os.path.isdir(_p) and _p not in sys.path:
        sys.path.insert(0, _p)

B, L, D, V, NL = 32, 40, 128, 32000, 2
NCORES = 8
CH = 8            # chains per core: 4 batch items x {q, a}
SEG = 2 * D + 2   # per-chain column span in the input tile: [0]=0, [1..128]=x, [129..256]=h, [257]=0
NF = CH * SEG

_CACHE = {}


def _build_nc(L=L):
    import concourse.bass as bass
    import concourse.bacc as bacc
    import concourse.mybir as mybir
    from concourse import tile

    f32 = mybir.dt.float32
    bf16 = mybir.dt.bfloat16
    fp8 = mybir.dt.float8e4
    i8 = mybir.dt.int8
    AF = mybir.ActivationFunctionType
    ALU = mybir.AluOpType
    DR = mybir.MatmulPerfMode.DoubleRow

    nc = bacc.Bacc(None, target_bir_lowering=False)

    dens_d = nc.dram_tensor("dens", (L, D, CH * D), fp8, kind="ExternalInput")
    st_d = nc.dram_tensor("st", (NL * 4 * 2, D, 2 * D), fp8, kind="ExternalInput")
    bias_d = nc.dram_tensor("bias", (D, NL * 4), f32, kind="ExternalInput")
    out_d = nc.dram_tensor("mp_out", (D, CH * D), bf16, kind="ExternalOutput")

    with tile.TileContext(nc) as tc:
        with (
            tc.tile_pool(name="const", bufs=1) as constp,
            tc.tile_pool(name="state", bufs=1) as statep,
            tc.tile_pool(name="gate", bufs=2) as gatep,
            tc.tile_pool(name="psum", bufs=1, space="PSUM") as psump,
        ):
            # ---- constants ----
            # stT[(l,g,pair)]: [K=128, plane(2) x M(128)] interleaved band pair
            stT = constp.tile([D, NL * 4 * 2 * 2 * D], fp8, tag="stT")
            bias = constp.tile([D, NL * 4], f32, tag="bias")

            # ---- persistent state ----
            c_l = [statep.tile([D, CH * D], bf16, tag=f"c{l}", name=f"c{l}") for l in range(NL)]
            mp = statep.tile([D, CH * D], bf16, tag="mp")

            I0 = [statep.tile([D, NF], fp8, tag=f"I0{p}", name=f"I0{p}") for p in range(2)]
            I1 = [statep.tile([D, NF], fp8, tag=f"I1{p}", name=f"I1{p}") for p in range(2)]

            def seg3(t):  # (p, s, c) view of an input tile
                return t[:].rearrange("p (s c) -> p s c", s=CH)

            def pairview(t):  # (p, two, s, j129) DoubleRow moving view
                # col = s*258 + j*2 + two;  plane dim (two) must be free dim 1
                return t[:].rearrange("p (s j two) -> p two s j", s=CH, two=2)

            # startup: density for step 0/1 first; constants spread over queues
            nc.sync.dma_start(seg3(I0[0])[:, :, 1:1 + D], dens_d[0])
            nc.sync.dma_start(bias[:], bias_d[:])
            qs = [nc.scalar, nc.sync]
            for i in range(NL * 4 * 2):
                qs[i % 2].dma_start(stT[:, i * 2 * D:(i + 1) * 2 * D], st_d[i])
            nc.sync.dma_start(seg3(I0[1])[:, :, 1:1 + D], dens_d[1])

            # warm the sigmoid/tanh spline tables while DMAs run
            warm = constp.tile([D, 2], f32, tag="warm")
            nc.scalar.activation(warm[:, 0:1], bias[:, 0:1], AF.Sigmoid)
            nc.scalar.activation(warm[:, 1:2], bias[:, 0:1], AF.Tanh)

            for l in range(NL):
                nc.vector.memset(c_l[l][:], 0.0)
            nc.vector.memset(mp[:], -1e30)
            for t_ in I0 + I1:
                v = seg3(t_)
                nc.gpsimd.memset(v[:, :, 0:1].bitcast(i8), 0)
                nc.gpsimd.memset(v[:, :, SEG - 1:SEG].bitcast(i8), 0)
            nc.gpsimd.memset(seg3(I0[0])[:, :, 129:129 + D].bitcast(i8), 0)  # h1_{-1}
            nc.gpsimd.memset(seg3(I1[0])[:, :, 129:129 + D].bitcast(i8), 0)  # h2_{-1}

            GTAG = ["pf", "pi", "po", "pc"]

            def emit_P(l, t):
                """One layer-step: matmul burst -> activations -> cell update
                -> route h to consumers."""
                inp = I0[t % 2] if l == 0 else I1[t % 2]
                pv = pairview(inp)
                ps = {g: psump.tile([D, CH * D], f32, tag=GTAG[g], name=GTAG[g])
                      for g in range(4)}
                # c-gate first: its activation is the head of the dependency
                # chain (cs -> t2 -> c -> th -> h)
                for g in (3, 0, 1, 2):
                    for half in range(2):
                        for pr in range(2):  # dh pairs (0,1) and (2,3)
                            idx = (l * 4 + g) * 2 + pr
                            lhsT = stT[:, idx * 2 * D:(idx + 1) * 2 * D] \
                                .rearrange("p (two m) -> p two m", two=2)
                            rhs = pv[:, :, half * 4:(half + 1) * 4, pr:pr + D]
                            nc.tensor.matmul(
                                ps[g][:, half * 512:(half + 1) * 512],
                                lhsT, rhs,
                                start=(pr == 0), stop=(pr == 1),
                                perf_mode=DR,
                            )
                # --- activations (bias folded in), chain-critical order ---
                fg = gatep.tile([D, CH * D], bf16, tag="fg")
                ig = gatep.tile([D, CH * D], bf16, tag="ig")
                og = gatep.tile([D, CH * D], bf16, tag="og")
                cs = gatep.tile([D, CH * D], bf16, tag="cs")
                t1 = gatep.tile([D, CH * D], bf16, tag="t1")
                t2 = gatep.tile([D, CH * D], bf16, tag="t2")
                th = gatep.tile([D, CH * D], bf16, tag="th")
                nc.scalar.activation(cs[:], ps[3][:], AF.Tanh,
                                     bias=bias[:, l * 4 + 3: l * 4 + 4])
                nc.scalar.activation(fg[:], ps[0][:], AF.Sigmoid,
                                     bias=bias[:, l * 4 + 0: l * 4 + 1])
                nc.scalar.activation(ig[:], ps[1][:], AF.Sigmoid,
                                     bias=bias[:, l * 4 + 1: l * 4 + 2])
                # --- cell update (all bf16 in SBUF: DVE 4x mode) ---
                nc.vector.tensor_mul(t1[:], fg[:], c_l[l][:])
                nc.vector.tensor_mul(t2[:], ig[:], cs[:])
                nc.vector.tensor_add(c_l[l][:], t1[:], t2[:])
                nc.scalar.activation(th[:], c_l[l][:], AF.Tanh)
                nc.scalar.activation(og[:], ps[2][:], AF.Sigmoid,
                                     bias=bias[:, l * 4 + 2: l * 4 + 3])
                # --- h = og * tanh(c): route to consumers (rounds to fp8) ---
                if l == 0:
                    if t + 1 < L:
                        # gates the very next P0 burst: emit first
                        nc.vector.tensor_mul(
                            seg3(I0[(t + 1) % 2])[:, :, 129:129 + D],
                            og[:], th[:])
                    nc.vector.tensor_mul(seg3(I1[t % 2])[:, :, 1:1 + D],
                                         og[:], th[:])
                else:
                    # bf16 copy for the running max (pooling stays bf16)
                    h2 = gatep.tile([D, CH * D], bf16, tag="h2")
                    nc.vector.tensor_mul(h2[:], og[:], th[:])
                    nc.vector.tensor_tensor(mp[:], mp[:], h2[:], op=ALU.max)
                    if t + 1 < L:
                        nc.vector.tensor_mul(
                            seg3(I1[(t + 1) % 2])[:, :, 129:129 + D],
                            og[:], th[:])

            # skewed schedule: PE alternates between the two layers' bursts
            emit_P(0, 0)
            for t in range(1, L):
                if t + 1 < L:
                    nc.sync.dma_start(seg3(I0[(t + 1) % 2])[:, :, 1:1 + D],
                                      dens_d[t + 1])
                emit_P(0, t)
                emit_P(1, t - 1)
            emit_P(1, L - 1)

            nc.sync.dma_start(out_d[:], mp[:])

    nc.compile()
    return nc


def _prep_core_inputs(dens_all, st, bias_arr, core):
    """dens_all: (B, 2, L, D, D) fp8 densities (axis1: 0=q, 1=a)."""
    sl = slice(4 * core, 4 * core + 4)
    ch = np.concatenate([dens_all[sl, 0], dens_all[sl, 1]], axis=0)  # (8, L, D, D)
    # dens[t, w, s*128+j] = ch[s, t, w, j]
    dens = np.ascontiguousarray(ch.transpose(1, 2, 0, 3)).reshape(L, D, CH * D)
    return {"dens": dens, "st": st, "bias": bias_arr}


def kernel(q, a, embed, conv_w, conv_b, lin_w, lin_b):
    import ml_dtypes
    from concourse import bass_utils

    fp8 = ml_dtypes.float8_e4m3
    q = np.asarray(q); a = np.asarray(a)
    embed = np.asarray(embed, np.float32)
    conv_w = np.asarray(conv_w, np.float32)
    conv_b = np.asarray(conv_b, np.float32)
    lin_w = np.asarray(lin_w, np.float32)
    lin_b = np.asarray(lin_b, np.float32)

    # host: embedding gather + density (normalized outer products)
    idx = np.stack([q, a], axis=1).astype(np.int64)            # (B, 2, L)
    xe = embed[idx].astype(np.float64)                         # (B, 2, L, D)
    dot = np.sum(xe * xe, axis=-1, keepdims=True) + 1e-4
    xe_y = (xe / np.sqrt(dot)).astype(np.float32)
    dens_all = np.einsum('bslw,bslj->bslwj', xe_y, xe_y).astype(fp8)

    # host: Toeplitz band stationaries, DoubleRow pair-interleaved:
    # st[(l,g,pair), k, plane*128 + m] = B_{2*pair+plane}^T[k, m],
    # B_dh[w, w'] = W[dh, w'-w+1]  (3 diagonals)
    st = np.zeros((NL * 4 * 2, D, 2 * D), np.float32)
    for l in range(NL):
        for g in range(4):
            W = conv_w[l, g, 0, 0]                             # (4, 3)
            for dh in range(4):
                Bm = sum(W[dh, dw] * np.eye(D, k=dw - 1) for dw in range(3))
                pr, pp = dh // 2, dh % 2
                st[(l * 4 + g) * 2 + pr, :, pp * D:(pp + 1) * D] = \
                    Bm.T.astype(np.float32)
    st = st.astype(fp8)
    bias_arr = np.tile(conv_b.reshape(1, -1), (D, 1)).astype(np.float32)

    if "nc" not in _CACHE:
        _CACHE["nc"] = _build_nc()
    nc = _CACHE["nc"]

    in_maps = [_prep_core_inputs(dens_all, st, bias_arr, i) for i in range(NCORES)]
    _CACHE["in_maps"] = in_maps
    res = bass_utils.run_bass_kernel_spmd(nc, in_maps, core_ids=list(range(NCORES)))

    # host: unshard + final linear + log_softmax
    q_p = np.zeros((B, D * D), np.float32)
    a_p = np.zeros((B, D * D), np.float32)
    for i in range(NCORES):
        out = np.asarray(res.results[i]["mp_out"]).astype(np.float32)  # (D w, CH*D)
        for s in range(CH):
            mp_T = out[:, s * D:(s + 1) * D]                   # (w, j)
            flat = np.ascontiguousarray(mp_T.T).reshape(-1)    # j-major
            if s < 4:
                q_p[4 * i + s] = flat
            else:
                a_p[4 * i + s - 4] = flat
    qa = np.concatenate([q_p, a_p], axis=1)
    score = qa @ lin_w.T + lin_b
    m = score.max(axis=1, keepdims=True)
    ls = score - m
    lse = np.log(np.exp(ls).sum(axis=1, keepdims=True))
    return (ls - lse).astype(np.float32)
